# revision 1
# baseline (speedup 1.0000x reference)
"""GCN layer (SpMM + Linear) on 8 Trainium2 NeuronCores.

out[i] = (sum_{e: row[e]==i} val[e] * X[col[e]]) @ W.T + b

Strategy:
- Destinations (rows of the output) are sharded across 8 cores
  (12500 rows each, padded to 12544 = 49 super-blocks of 256 dests).
- Edges are partitioned by destination core, grouped by
  (dest super-block, source chunk) where source chunks are 4 x 25000
  rows of X (so chunk-local column indices fit in int16).
- X is pre-cast to fp16 on the host. For each group, the edge source
  rows are gathered from HBM via dma_gather (128 edges -> 128
  partitions), giving msgs tiles [128 edges, nb, 256 feat].
- Aggregation via one-hot matmul: O[e, d] = val[e] * (row_local[e]==d)
  built on DVE with a single tensor_scalar (iota == row) * val, then
  PE matmuls psum_hT[f_half, dest] += msgs_half.T @ O accumulated over
  all batches of a super-block.
- The Linear layer runs on-chip: out[dest, fo] = sum_f hT[f, d]*W.T[f, fo]
  as two fp32 matmuls per 128-dest block. Bias is added on the host.

Group capacities are static per (super, chunk) = max edge count over
the 8 cores rounded up to 128; cores pad with (idx=0, val=0) edges so
the single SPMD program is identical across cores.
"""

import math
from contextlib import ExitStack

import numpy as np

N_NODES = 100000
N_EDGES = 3200000
D = 256
NCORES = 8

_PROGRAM_CACHE = {}


def _patch_tile_drain():
    """Split end-of-kernel drain waits into 1-sem carrier nops.

    The walrus build in this container rejects TPB_CTRL instructions
    with more than one sync wait ("Too many sync wait commands"); Tile's
    stock _drain_and_barrier puts the whole global clock on one drain.
    """
    import concourse.tile as tile
    from concourse.vector_clock import ScopedClock, VectorClock

    if getattr(tile.TileContext, "_drain_patched", False):
        return

    def _drain_and_barrier(self, tick_clock, wait_clock):
        nc = self.nc
        vc = tick_clock.global_clock
        for p in range(len(vc)):
            if vc[p] > 0:
                sub = VectorClock()
                sub.require_at_least(p, vc[p])
                carrier = nc.sync.nop()
                wait_clock.add_sem_waits(carrier.ins, ScopedClock({None: sub}))
        nc.sync.drain()
        nc.all_engine_barrier()
        assert self.sems is not None
        popped = nc._tile_sem_poison_stack.pop()
        assert popped is self._sem_poison
        nc.clear_and_free_semaphores(list(self.sems.allocated().values()))
        nc.all_engine_barrier()

    tile.TileContext._drain_and_barrier = _drain_and_barrier
    tile.TileContext._drain_patched = True


def _plan(edge_row, edge_col, n_nodes, ncores, super_w, n_chunks):
    """Static group plan shared by all cores.

    Returns caps[n_supers * n_chunks] (padded edge counts per
    (super, chunk) group, identical across cores) plus per-edge group
    assignment arrays.
    """
    rows_per_core = n_nodes // ncores
    n_supers = math.ceil(rows_per_core / super_w)
    chunk_sz = n_nodes // n_chunks

    core = edge_row // rows_per_core
    r_local = edge_row - core * rows_per_core
    sup = r_local // super_w
    chunk = edge_col // chunk_sz
    gid = sup * n_chunks + chunk
    n_groups = n_supers * n_chunks

    counts = np.zeros((ncores, n_groups), np.int64)
    np.add.at(counts, (core, gid), 1)
    caps = counts.max(axis=0)
    caps = np.maximum(((caps + 127) // 128) * 128, 128)
    return caps, core, r_local, sup, chunk, gid, n_supers, chunk_sz


def _pack_core(k, caps, core, r_local, sup, chunk, gid, edge_col, edge_val,
               super_w, chunk_sz, n_chunks):
    """Build the packed int16 idx+meta plane [128, TOT_COLS] for core k."""
    n_groups = len(caps)
    sel = np.flatnonzero(core == k)
    g = gid[sel]
    order = np.argsort(g, kind="stable")
    sel = sel[order]
    g = g[order]

    # position of each edge inside the padded flat layout
    cap_off = np.zeros(n_groups + 1, np.int64)
    np.cumsum(caps, out=cap_off[1:])
    grp_start = np.searchsorted(g, np.arange(n_groups))
    rank = np.arange(len(g)) - grp_start[g]
    pos = cap_off[g] + rank

    total = int(cap_off[-1])
    lc = np.zeros(total, np.int16)
    rl = np.zeros(total, np.float32)
    vv = np.zeros(total, np.float32)
    lc[pos] = (edge_col[sel] - chunk[sel] * chunk_sz).astype(np.int16)
    rl[pos] = (r_local[sel] - sup[sel] * super_w).astype(np.float32)
    vv[pos] = edge_val[sel].astype(np.float32)

    planes = []
    for gi in range(n_groups):
        a, b = int(cap_off[gi]), int(cap_off[gi + 1])
        cap = b - a
        nb = cap // 128
        # idx: wrapped in 16 partitions, replicated 8x to 128
        w16 = lc[a:b].reshape(cap // 16, 16).T  # [16, cap/16]
        idx_plane = np.tile(w16, (8, 1))  # [128, cap/16] int16
        # meta: [128, 2*nb] fp32 (row, val per batch) -> int16 bits
        meta = np.empty((128, 2 * nb), np.float32)
        meta[:, 0::2] = rl[a:b].reshape(nb, 128).T
        meta[:, 1::2] = vv[a:b].reshape(nb, 128).T
        planes.append(idx_plane)
        planes.append(
            meta.view(np.int16).reshape(128, 4 * nb))
    return np.ascontiguousarray(np.concatenate(planes, axis=1))


def _build_program(caps, n_nodes, super_w, n_supers, n_chunks, chunk_sz,
                   mode="full"):
    import concourse.bacc as bacc
    import concourse.mybir as mybir
    import concourse.tile as tile

    fp16 = mybir.dt.float16
    fp32 = mybir.dt.float32
    int16 = mybir.dt.int16
    n_groups = len(caps)
    rows_pad = n_supers * super_w

    # column offsets of idx and meta sections per group in the packed plane
    idx_off = np.zeros(n_groups, np.int64)
    meta_off = np.zeros(n_groups, np.int64)
    o = 0
    for gi in range(n_groups):
        cap = int(caps[gi])
        idx_off[gi] = o
        o += cap // 16
        meta_off[gi] = o
        o += 4 * (cap // 128)
    tot_cols = o

    nc = bacc.Bacc("TRN2", target_bir_lowering=False)
    X16 = nc.dram_tensor("x16", [n_nodes, D], fp16, kind="ExternalInput")
    IM = nc.dram_tensor("idxmeta", [128, tot_cols], int16, kind="ExternalInput")
    IOTA = nc.dram_tensor("iota", [128, super_w], fp16, kind="ExternalInput")
    WT = nc.dram_tensor("wt", [D, D], fp32, kind="ExternalInput")
    OUT = nc.dram_tensor("out", [rows_pad, D], fp32, kind="ExternalOutput")

    with tile.TileContext(nc) as tc, ExitStack() as ctx:
        const_pool = ctx.enter_context(tc.tile_pool(name="const", bufs=1))
        msgs_pool = ctx.enter_context(tc.tile_pool(name="msgs", bufs=3))
        o_pool = ctx.enter_context(tc.tile_pool(name="onehot", bufs=8))
        h_pool = ctx.enter_context(tc.tile_pool(name="h", bufs=2))
        out_pool = ctx.enter_context(tc.tile_pool(name="outp", bufs=3))
        psum_pool = ctx.enter_context(
            tc.tile_pool(name="psum", bufs=2, space="PSUM"))
        psum_out_pool = ctx.enter_context(
            tc.tile_pool(name="psum_out", bufs=2, space="PSUM"))

        im_t = const_pool.tile([128, tot_cols], int16)
        nc.sync.dma_start(im_t[:], IM[:])
        iota_t = const_pool.tile([128, super_w], fp16)
        nc.sync.dma_start(iota_t[:], IOTA[:])
        wt_t = const_pool.tile([128, 2, D], fp32)
        nc.sync.dma_start(wt_t[:, 0, :], WT[0:128, :])
        nc.sync.dma_start(wt_t[:, 1, :], WT[128:256, :])

        for s in range(n_supers):
            if mode == "nomm":
                h0 = h_pool.tile([128, super_w], fp32, tag="h0")
                h1 = h_pool.tile([128, super_w], fp32, tag="h1")
                nc.vector.memset(h0[:], 0.0)
                nc.vector.memset(h1[:], 0.0)
                for bb in range(super_w // 128):
                    po = psum_out_pool.tile([128, D], fp32, tag="po")
                    nc.tensor.matmul(po[:], h0[:, bb * 128:(bb + 1) * 128],
                                     wt_t[:, 0, :], start=True, stop=False)
                    nc.tensor.matmul(po[:], h1[:, bb * 128:(bb + 1) * 128],
                                     wt_t[:, 1, :], start=False, stop=True)
                    ot = out_pool.tile([128, D], fp32, tag="ot")
                    nc.scalar.copy(ot[:], po[:])
                    nc.sync.dma_start(
                        OUT[s * super_w + bb * 128:
                            s * super_w + (bb + 1) * 128, :], ot[:])
                continue
            pT0 = psum_pool.tile([128, super_w], fp32, tag="p0")
            pT1 = psum_pool.tile([128, super_w], fp32, tag="p1")
            first = True
            for c in range(n_chunks):
                gi = s * n_chunks + c
                cap = int(caps[gi])
                nb = cap // 128
                mt = msgs_pool.tile([128, nb, D], fp16, tag="msgs")
                if mode == "nogather":
                    nc.vector.memset(mt[:], 0.0)
                else:
                    nc.gpsimd.dma_gather(
                        mt[:],
                        X16[c * chunk_sz:(c + 1) * chunk_sz, :],
                        im_t[:, int(idx_off[gi]):int(idx_off[gi]) + cap // 16],
                        cap,
                        cap,
                        D,
                        elem_step=D,
                        single_packet=(cap <= 1024),
                    )
                for j in range(nb):
                    mo = int(meta_off[gi]) + 4 * j
                    oh = o_pool.tile([128, super_w], fp16, tag="oh")
                    if mode == "noonehot":
                        nc.vector.memset(oh[:], 0.0)
                    else:
                        nc.vector.tensor_scalar(
                            oh[:],
                            iota_t[:],
                            im_t[:, mo:mo + 2].bitcast(fp32),
                            im_t[:, mo + 2:mo + 4].bitcast(fp32),
                            mybir.AluOpType.is_equal,
                            mybir.AluOpType.mult,
                        )
                    last = (c == n_chunks - 1) and (j == nb - 1)
                    nc.tensor.matmul(pT0[:], mt[:, j, 0:128], oh[:],
                                     start=first, stop=last)
                    if mode != "onehalf":
                        nc.tensor.matmul(pT1[:], mt[:, j, 128:256], oh[:],
                                         start=first, stop=last)
                    first = False

            h0 = h_pool.tile([128, super_w], fp32, tag="h0")
            h1 = h_pool.tile([128, super_w], fp32, tag="h1")
            nc.scalar.copy(h0[:], pT0[:])
            nc.scalar.copy(h1[:], pT0[:] if mode == "onehalf" else pT1[:])
            for bb in range(super_w // 128):
                if mode == "noW":
                    ot = out_pool.tile([128, D], fp32, tag="ot")
                    nc.vector.tensor_copy(
                        ot[:, 0:128], h0[:, bb * 128:(bb + 1) * 128])
                    nc.vector.tensor_copy(
                        ot[:, 128:256], h1[:, bb * 128:(bb + 1) * 128])
                else:
                    po = psum_out_pool.tile([128, D], fp32, tag="po")
                    nc.tensor.matmul(po[:], h0[:, bb * 128:(bb + 1) * 128],
                                     wt_t[:, 0, :], start=True, stop=False)
                    nc.tensor.matmul(po[:], h1[:, bb * 128:(bb + 1) * 128],
                                     wt_t[:, 1, :], start=False, stop=True)
                    ot = out_pool.tile([128, D], fp32, tag="ot")
                    nc.scalar.copy(ot[:], po[:])
                nc.sync.dma_start(
                    OUT[s * super_w + bb * 128:s * super_w + (bb + 1) * 128, :],
                    ot[:])
    nc.finalize()
    return nc


def _prepare(X, edge_row, edge_col, edge_val, W,
             n_nodes, ncores, super_w, n_chunks):
    X = np.asarray(X)
    edge_row = np.asarray(edge_row)
    edge_col = np.asarray(edge_col)
    edge_val = np.asarray(edge_val)
    W = np.asarray(W)

    caps, core, r_local, sup, chunk, gid, n_supers, chunk_sz = _plan(
        edge_row, edge_col, n_nodes, ncores, super_w, n_chunks)

    key = (n_nodes, ncores, super_w, n_chunks, tuple(caps.tolist()))
    if key not in _PROGRAM_CACHE:
        _PROGRAM_CACHE[key] = _build_program(
            caps, n_nodes, super_w, n_supers, n_chunks, chunk_sz)
    nc = _PROGRAM_CACHE[key]

    X16 = np.ascontiguousarray(X.astype(np.float16))
    iota = np.tile(np.arange(super_w, dtype=np.float16), (128, 1))
    wt = np.ascontiguousarray(W.T.astype(np.float32))

    in_maps = []
    for k in range(ncores):
        im = _pack_core(k, caps, core, r_local, sup, chunk, gid,
                        edge_col, edge_val, super_w, chunk_sz, n_chunks)
        in_maps.append({"x16": X16, "idxmeta": im, "iota": iota, "wt": wt})
    return nc, in_maps


def _gather_out(res, b, n_nodes, ncores):
    rows_per_core = n_nodes // ncores
    out = np.empty((n_nodes, D), np.float32)
    for k in range(ncores):
        out[k * rows_per_core:(k + 1) * rows_per_core] = \
            res.results[k]["out"][:rows_per_core]
    out += np.asarray(b).astype(np.float32)[None, :]
    return out


def _run(X, edge_row, edge_col, edge_val, W, b,
         n_nodes, ncores, super_w, n_chunks):
    from concourse.bass_utils import run_bass_kernel_spmd

    nc, in_maps = _prepare(X, edge_row, edge_col, edge_val, W,
                           n_nodes, ncores, super_w, n_chunks)
    res = run_bass_kernel_spmd(nc, in_maps, core_ids=list(range(ncores)))
    return _gather_out(res, b, n_nodes, ncores)


def kernel(X, edge_row, edge_col, edge_val, W, b):
    return _run(X, edge_row, edge_col, edge_val, W, b,
                n_nodes=N_NODES, ncores=NCORES, super_w=256, n_chunks=4)


def run_traced(X, edge_row, edge_col, edge_val, W, b):
    """Run with NTFF profiling; returns BassKernelResults."""
    from concourse.bass_utils import run_bass_kernel_spmd

    nc, in_maps = _prepare(X, edge_row, edge_col, edge_val, W,
                           n_nodes=N_NODES, ncores=NCORES, super_w=256,
                           n_chunks=4)
    return run_bass_kernel_spmd(nc, in_maps, core_ids=list(range(NCORES)),
                                trace=True)



# revision 13
# speedup vs baseline: 2.8521x; 2.8521x over previous
"""GCN layer (SpMM + Linear) on 8 Trainium2 NeuronCores.

out[i] = (sum_{e: row[e]==i} val[e] * X[col[e]]) @ W.T + b

v2 strategy (per core; destinations sharded across 8 cores):
- Dest rows sharded: 12500 rows/core, padded to 12544 = 98 supers of 128.
- Edges grouped by (dest super, source chunk); 4 source chunks of 25000
  rows so chunk-local gather indices fit in int16.
- Gather: gpsimd.dma_gather of X fp16 rows (512B/edge) spread over all
  4 SWDGE queues (queue = chunk) so descriptor generation runs on all
  8 Q7 cores in parallel.  Padding uses trailing -1 indices, which the
  Q7 ucode strips, so pad slots cost no descriptors.
- One-hot: per super, ONE is_equal + ONE mult DVE tensor_tensor over
  [128 edges, 128 dests, nb batches] (dest-major, batch-minor layout so
  every operand has stride-1 last dim -> DVE 2x perf mode).
- Aggregation: per batch j, matmul(psum[dest,feat] += oh_j.T @ msgs_j)
  with the 128x128 one-hot stationary and 256-wide msgs streaming.
- Linear: psum -> fp16, PE-transpose both halves, 2 fp16 matmuls with
  W.T resident; bias added on host.
"""

import math
import os
from contextlib import ExitStack

import numpy as np

# debug knobs
N_QUEUES = int(os.environ.get("GCN_N_QUEUES", "4"))
PAD_NEG1 = os.environ.get("GCN_PAD_NEG1", "0") == "1"

N_NODES = 100000
N_EDGES = 3200000
D = 256
NCORES = 8
SUPER_W = 128
N_CHUNKS = 4

_PROGRAM_CACHE = {}


def _patch_tile_drain():
    """Split end-of-kernel drain waits into 1-sem carrier nops.

    The walrus build in this container rejects TPB_CTRL instructions
    with more than one sync wait ("Too many sync wait commands"); Tile's
    stock _drain_and_barrier puts the whole global clock on one drain.
    """
    import concourse.tile as tile
    from concourse.vector_clock import ScopedClock, VectorClock

    if getattr(tile.TileContext, "_drain_patched", False):
        return

    def _drain_and_barrier(self, tick_clock, wait_clock):
        nc = self.nc
        vc = tick_clock.global_clock
        for p in range(len(vc)):
            if vc[p] > 0:
                sub = VectorClock()
                sub.require_at_least(p, vc[p])
                carrier = nc.sync.nop()
                wait_clock.add_sem_waits(carrier.ins, ScopedClock({None: sub}))
        nc.sync.drain()
        nc.all_engine_barrier()
        assert self.sems is not None
        popped = nc._tile_sem_poison_stack.pop()
        assert popped is self._sem_poison
        nc.clear_and_free_semaphores(list(self.sems.allocated().values()))
        nc.all_engine_barrier()

    tile.TileContext._drain_and_barrier = _drain_and_barrier
    tile.TileContext._drain_patched = True


def _plan(edge_row, edge_col, n_nodes, ncores):
    """Static plan shared by all cores.

    caps[s, c]: per-(super, chunk) edge capacity (max count over cores,
    rounded up to 128).  nbs_alloc[s]: batches per super (even, >= 2).
    """
    rows_per_core = n_nodes // ncores
    n_supers = math.ceil(rows_per_core / SUPER_W)
    chunk_sz = n_nodes // N_CHUNKS

    core = edge_row // rows_per_core
    r_local = edge_row - core * rows_per_core
    sup = r_local // SUPER_W
    chunk = edge_col // chunk_sz
    gid = sup * N_CHUNKS + chunk
    n_groups = n_supers * N_CHUNKS

    counts = np.zeros((ncores, n_groups), np.int64)
    np.add.at(counts, (core, gid), 1)
    caps = counts.max(axis=0)
    caps = np.maximum(((caps + 127) // 128) * 128, 128).reshape(
        n_supers, N_CHUNKS)

    nbs = caps.sum(axis=1) // 128
    nbs_alloc = nbs + (nbs % 2)
    return (caps, nbs_alloc, core, r_local, sup, chunk, gid, n_supers,
            chunk_sz)


def _pack_core(k, caps, nbs_alloc, core, r_local, sup, chunk, gid,
               edge_col, edge_val, chunk_sz, first_full_groups):
    """Build per-core packed planes.

    Returns (idx_plane [128, IDX_COLS] int16, meta [128, 2*META_COLS] fp16).
    idx: per (super, chunk) group, chunk-local cols wrapped in 16
    partitions, replicated 8x; pad = -1 (trailing; skipped by ucode) or
    0 for the first few groups so every msgs slot sees finite data once.
    meta: per super, rows fp16 then vals fp16, [128, nbs_alloc] each.
    """
    n_supers, n_chunks = caps.shape
    sel = np.flatnonzero(core == k)
    g = gid[sel]
    order = np.argsort(g, kind="stable")
    sel = sel[order]
    g = g[order]

    caps_flat = caps.reshape(-1)
    cap_off = np.zeros(n_supers * n_chunks + 1, np.int64)
    np.cumsum(caps_flat, out=cap_off[1:])
    grp_start = np.searchsorted(g, np.arange(n_supers * n_chunks))
    rank = np.arange(len(g)) - grp_start[g]
    pos = cap_off[g] + rank

    total = int(cap_off[-1])
    lc = np.full(total, -1 if PAD_NEG1 else 0, np.int16)
    rl = np.zeros(total, np.float16)
    vv = np.zeros(total, np.float16)
    lc[pos] = (edge_col[sel] - chunk[sel] * chunk_sz).astype(np.int16)
    rl[pos] = (r_local[sel] - sup[sel] * SUPER_W).astype(np.float16)
    vv[pos] = edge_val[sel].astype(np.float16)

    idx_planes = []
    gi = 0
    for s in range(n_supers):
        for c in range(n_chunks):
            a, b = int(cap_off[gi]), int(cap_off[gi + 1])
            cap = b - a
            li = lc[a:b].copy()
            if gi < first_full_groups:
                li[li < 0] = 0
            w16 = li.reshape(cap // 16, 16).T  # [16, cap/16]
            idx_planes.append(np.tile(w16, (8, 1)))  # [128, cap/16]
            gi += 1
    idx_plane = np.ascontiguousarray(np.concatenate(idx_planes, axis=1))

    meta_planes = []
    for s in range(n_supers):
        a = int(cap_off[s * n_chunks])
        b = int(cap_off[(s + 1) * n_chunks])
        nb = (b - a) // 128
        nba = int(nbs_alloc[s])
        rows = np.zeros((128, nba), np.float16)
        vals = np.zeros((128, nba), np.float16)
        rows[:, :nb] = rl[a:b].reshape(nb, 128).T
        vals[:, :nb] = vv[a:b].reshape(nb, 128).T
        meta_planes.append(rows)
        meta_planes.append(vals)
    meta = np.ascontiguousarray(np.concatenate(meta_planes, axis=1))
    return idx_plane, meta


def _build_program(caps, nbs_alloc, n_nodes, n_supers, chunk_sz,
                   mode="full"):
    import concourse.bacc as bacc
    import concourse.mybir as mybir
    import concourse.tile as tile

    _patch_tile_drain()

    fp16 = mybir.dt.float16
    fp32 = mybir.dt.float32
    int16 = mybir.dt.int16
    n_chunks = caps.shape[1]
    rows_pad = n_supers * SUPER_W
    nb_grp = caps // 128          # batches per (super, chunk)
    nba_max = int(nbs_alloc.max())
    msgs_nb_max = int(nb_grp.max())

    # idx column offsets per group; meta column offsets per super
    idx_off = np.zeros((n_supers, n_chunks), np.int64)
    o = 0
    for s in range(n_supers):
        for c in range(n_chunks):
            idx_off[s, c] = o
            o += int(caps[s, c]) // 16
    idx_cols = int(o)
    meta_off = np.zeros(n_supers, np.int64)
    o = 0
    for s in range(n_supers):
        meta_off[s] = o
        o += 2 * int(nbs_alloc[s])
    meta_cols = int(o)

    nc = bacc.Bacc("TRN2", target_bir_lowering=False,
                   num_swdge_queues=N_QUEUES)
    X16 = nc.dram_tensor("x16", [n_nodes, D], fp16, kind="ExternalInput")
    IDX = nc.dram_tensor("idx", [128, idx_cols], int16, kind="ExternalInput")
    META = nc.dram_tensor("meta", [128, meta_cols], fp16,
                          kind="ExternalInput")
    IOTA = nc.dram_tensor("iota", [128, SUPER_W, nba_max], fp16,
                          kind="ExternalInput")
    IDENT = nc.dram_tensor("ident", [128, 128], fp16, kind="ExternalInput")
    WT = nc.dram_tensor("wt", [D, D], fp16, kind="ExternalInput")
    OUT = nc.dram_tensor("out", [rows_pad, D], fp32, kind="ExternalOutput")

    with tile.TileContext(nc) as tc, ExitStack() as ctx:
        const_pool = ctx.enter_context(tc.tile_pool(name="const", bufs=1))
        msgs_pool = ctx.enter_context(tc.tile_pool(name="msgs", bufs=6))
        oh_pool = ctx.enter_context(tc.tile_pool(name="oh", bufs=3))
        h_pool = ctx.enter_context(tc.tile_pool(name="h", bufs=2))
        ht_pool = ctx.enter_context(tc.tile_pool(name="ht", bufs=4))
        out_pool = ctx.enter_context(tc.tile_pool(name="outp", bufs=3))
        psum_pool = ctx.enter_context(
            tc.tile_pool(name="psum", bufs=2, space="PSUM"))
        psum_t_pool = ctx.enter_context(
            tc.tile_pool(name="psum_t", bufs=2, space="PSUM"))
        psum_o_pool = ctx.enter_context(
            tc.tile_pool(name="psum_o", bufs=2, space="PSUM"))

        idx_t = const_pool.tile([128, idx_cols], int16)
        nc.sync.dma_start(idx_t[:], IDX[:])
        meta_t = const_pool.tile([128, meta_cols], fp16)
        nc.sync.dma_start(meta_t[:], META[:])
        iota_t = const_pool.tile([128, SUPER_W, nba_max], fp16)
        nc.sync.dma_start(iota_t[:], IOTA[:])
        ident_t = const_pool.tile([128, 128], fp16)
        nc.sync.dma_start(ident_t[:], IDENT[:])
        wt_t = const_pool.tile([128, 2, D], fp16)
        nc.sync.dma_start(wt_t[:, 0, :], WT[0:128, :])
        nc.sync.dma_start(wt_t[:, 1, :], WT[128:256, :])

        for s in range(n_supers):
            nba = int(nbs_alloc[s])
            # --- batched one-hot for the whole super ---
            oh_t = oh_pool.tile([128, SUPER_W, nba_max], fp16, tag="oh")
            if mode == "noonehot":
                pass
            else:
                mo = int(meta_off[s])
                row_ap = meta_t[:, mo:mo + nba].unsqueeze(1).broadcast_to(
                    [128, SUPER_W, nba])
                val_ap = meta_t[:, mo + nba:mo + 2 * nba].unsqueeze(
                    1).broadcast_to([128, SUPER_W, nba])
                nc.vector.tensor_tensor(
                    oh_t[:, :, 0:nba], iota_t[:, :, 0:nba], row_ap,
                    mybir.AluOpType.is_equal)
                nc.vector.tensor_tensor(
                    oh_t[:, :, 0:nba], oh_t[:, :, 0:nba], val_ap,
                    mybir.AluOpType.mult)

            # --- gather + aggregate ---
            pT = psum_pool.tile([128, D], fp32, tag="ps")
            jj = 0
            first = True
            for c in range(n_chunks):
                cap = int(caps[s, c])
                nb = cap // 128
                mt = msgs_pool.tile([128, msgs_nb_max, D], fp16, tag="msgs")
                if s * n_chunks + c < 6:
                    # first use of each of the 6 msgs slots: clear the whole
                    # slot so pad rows (skipped by the -1-idx gather) always
                    # hold finite values for the val=0 one-hot columns
                    nc.vector.memset(mt[:], 0.0)
                if mode == "nogather":
                    pass
                else:
                    io = int(idx_off[s, c])
                    nc.gpsimd.dma_gather(
                        mt[:, 0:nb, :],
                        X16[c * chunk_sz:(c + 1) * chunk_sz, :],
                        idx_t[:, io:io + cap // 16],
                        cap,
                        cap,
                        D,
                        elem_step=D,
                        single_packet=(cap <= 1024),
                        queue_num=c % N_QUEUES,
                    )
                for j in range(nb):
                    last = (c == n_chunks - 1) and (j == nb - 1)
                    if mode == "nomm":
                        jj += 1
                        first = False
                        continue
                    nc.tensor.matmul(pT[:], oh_t[:, :, jj], mt[:, j, :],
                                     start=first, stop=last)
                    jj += 1
                    first = False
            if mode == "nomm":
                continue

            # --- linear: psum[d,f] -> hT -> @ W.T ---
            hs = h_pool.tile([128, D], fp16, tag="hs")
            nc.scalar.copy(hs[:], pT[:])
            po = psum_o_pool.tile([128, D], fp32, tag="po")
            if mode == "noW":
                ot = out_pool.tile([128, D], fp32, tag="ot")
                nc.scalar.copy(ot[:], hs[:])
            else:
                for hh in range(2):
                    ptr = psum_t_pool.tile([128, 128], fp16, tag="ptr")
                    nc.tensor.transpose(
                        ptr[:], hs[:, hh * 128:(hh + 1) * 128], ident_t[:])
                    ht = ht_pool.tile([128, 128], fp16, tag="ht")
                    nc.scalar.copy(ht[:], ptr[:])
                    nc.tensor.matmul(po[:], ht[:], wt_t[:, hh, :],
                                     start=(hh == 0), stop=(hh == 1))
                ot = out_pool.tile([128, D], fp32, tag="ot")
                nc.scalar.copy(ot[:], po[:])
            nc.sync.dma_start(
                OUT[s * SUPER_W:(s + 1) * SUPER_W, :], ot[:])
    nc.finalize()
    return nc


def _prepare(X, edge_row, edge_col, edge_val, W, mode="full"):
    X = np.asarray(X)
    edge_row = np.asarray(edge_row)
    edge_col = np.asarray(edge_col)
    edge_val = np.asarray(edge_val)
    W = np.asarray(W)

    (caps, nbs_alloc, core, r_local, sup, chunk, gid, n_supers,
     chunk_sz) = _plan(edge_row, edge_col, N_NODES, NCORES)

    key = (mode, tuple(caps.reshape(-1).tolist()))
    if key not in _PROGRAM_CACHE:
        _PROGRAM_CACHE[key] = _build_program(
            caps, nbs_alloc, N_NODES, n_supers, chunk_sz, mode=mode)
    nc = _PROGRAM_CACHE[key]

    nba_max = int(nbs_alloc.max())
    X16 = np.ascontiguousarray(X.astype(np.float16))
    iota = np.ascontiguousarray(np.broadcast_to(
        np.arange(SUPER_W, dtype=np.float16)[None, :, None],
        (128, SUPER_W, nba_max)))
    ident = np.eye(128, dtype=np.float16)
    wt = np.ascontiguousarray(W.T.astype(np.float16))

    in_maps = []
    for k in range(NCORES):
        idx_plane, meta = _pack_core(
            k, caps, nbs_alloc, core, r_local, sup, chunk, gid,
            edge_col, edge_val, chunk_sz, first_full_groups=0)
        in_maps.append({"x16": X16, "idx": idx_plane, "meta": meta,
                        "iota": iota, "ident": ident, "wt": wt})
    return nc, in_maps


def _gather_out(res, b):
    rows_per_core = N_NODES // NCORES
    out = np.empty((N_NODES, D), np.float32)
    for k in range(NCORES):
        out[k * rows_per_core:(k + 1) * rows_per_core] = \
            res.results[k]["out"][:rows_per_core]
    out += np.asarray(b).astype(np.float32)[None, :]
    return out


def kernel(X, edge_row, edge_col, edge_val, W, b):
    from concourse.bass_utils import run_bass_kernel_spmd

    nc, in_maps = _prepare(X, edge_row, edge_col, edge_val, W)
    res = run_bass_kernel_spmd(nc, in_maps, core_ids=list(range(NCORES)))
    return _gather_out(res, b)


def run_traced(X, edge_row, edge_col, edge_val, W, b, mode="full"):
    """Run with NTFF profiling; returns BassKernelResults."""
    from concourse.bass_utils import run_bass_kernel_spmd

    nc, in_maps = _prepare(X, edge_row, edge_col, edge_val, W, mode=mode)
    return run_bass_kernel_spmd(nc, in_maps, core_ids=list(range(NCORES)),
                                trace=True)


# revision 29
# speedup vs baseline: 3.2493x; 1.1393x over previous
"""GCN layer (SpMM + Linear) on 8 Trainium2 NeuronCores.

out[i] = (sum_{e: row[e]==i} val[e] * X[col[e]]) @ W.T + b

v2 strategy (per core; destinations sharded across 8 cores):
- Dest rows sharded: 12500 rows/core, padded to 12544 = 98 supers of 128.
- Edges grouped by (dest super, source chunk); 4 source chunks of 25000
  rows so chunk-local gather indices fit in int16.
- Gather: gpsimd.dma_gather of X fp16 rows (512B/edge) spread over all
  4 SWDGE queues (queue = chunk) so descriptor generation runs on all
  8 Q7 cores in parallel.  Padding uses trailing -1 indices, which the
  Q7 ucode strips, so pad slots cost no descriptors.
- One-hot: per super, ONE is_equal + ONE mult DVE tensor_tensor over
  [128 edges, 128 dests, nb batches] (dest-major, batch-minor layout so
  every operand has stride-1 last dim -> DVE 2x perf mode).
- Aggregation: per batch j, matmul(psum[dest,feat] += oh_j.T @ msgs_j)
  with the 128x128 one-hot stationary and 256-wide msgs streaming.
- Linear: psum -> fp16, PE-transpose both halves, 2 fp16 matmuls with
  W.T resident; bias added on host.
"""

import math
import os
from contextlib import ExitStack

import numpy as np

# debug knobs
N_QUEUES = int(os.environ.get("GCN_N_QUEUES", "4"))
PAD_NEG1 = os.environ.get("GCN_PAD_NEG1", "1") == "1"
DYN_REG = os.environ.get("GCN_DYN_REG", "1") == "1"
BALANCE = os.environ.get("GCN_BALANCE", "1") == "1"

N_NODES = 100000
N_EDGES = 3200000
D = 256
NCORES = 8
SUPER_W = 128
N_CHUNKS = 4

_PROGRAM_CACHE = {}


def _patch_tile_drain():
    """Split end-of-kernel drain waits into 1-sem carrier nops.

    The walrus build in this container rejects TPB_CTRL instructions
    with more than one sync wait ("Too many sync wait commands"); Tile's
    stock _drain_and_barrier puts the whole global clock on one drain.
    """
    import concourse.tile as tile
    from concourse.vector_clock import ScopedClock, VectorClock

    if getattr(tile.TileContext, "_drain_patched", False):
        return

    def _drain_and_barrier(self, tick_clock, wait_clock):
        nc = self.nc
        vc = tick_clock.global_clock
        for p in range(len(vc)):
            if vc[p] > 0:
                sub = VectorClock()
                sub.require_at_least(p, vc[p])
                carrier = nc.sync.nop()
                wait_clock.add_sem_waits(carrier.ins, ScopedClock({None: sub}))
        nc.sync.drain()
        nc.all_engine_barrier()
        assert self.sems is not None
        popped = nc._tile_sem_poison_stack.pop()
        assert popped is self._sem_poison
        nc.clear_and_free_semaphores(list(self.sems.allocated().values()))
        nc.all_engine_barrier()

    tile.TileContext._drain_and_barrier = _drain_and_barrier
    tile.TileContext._drain_patched = True


def _plan(edge_row, edge_col, n_nodes, ncores):
    """Static plan shared by all cores.

    Each core assigns its 12500 dest rows to (super, slot) pairs,
    balancing per-super edge counts (snake deal by degree).  caps[s, c]:
    per-(super, chunk) capacity (max true count over cores, rounded up
    to 128).  Returns per-edge sup/slot/chunk plus per-core true counts
    and the dest_of[core, sup*128+slot] -> r_local map for unpermuting.
    """
    rows_per_core = n_nodes // ncores
    n_supers = math.ceil(rows_per_core / SUPER_W)
    rows_pad = n_supers * SUPER_W
    chunk_sz = n_nodes // N_CHUNKS

    core = edge_row // rows_per_core
    r_local = edge_row - core * rows_per_core
    chunk = edge_col // chunk_sz

    sup_of = np.zeros((ncores, rows_per_core), np.int32)
    slot_of = np.zeros((ncores, rows_per_core), np.int32)
    dest_of = np.full((ncores, rows_pad), -1, np.int64)
    if BALANCE:
        deg = np.zeros((ncores, rows_per_core), np.int64)
        np.add.at(deg, (core, r_local), 1)
        for k in range(ncores):
            order = np.argsort(-deg[k], kind="stable")
            # snake deal across supers: 0..S-1, S-1..0, ...
            fwd = np.arange(n_supers)
            snake = np.concatenate([fwd, fwd[::-1]])
            sup_seq = np.resize(snake, rows_per_core)
            s_assign = np.empty(rows_per_core, np.int32)
            s_assign[order] = sup_seq
            sup_of[k] = s_assign
            # slot = rank within super
            slot = np.zeros(rows_per_core, np.int32)
            for s in range(n_supers):
                idxs = np.flatnonzero(s_assign == s)
                slot[idxs] = np.arange(len(idxs))
            slot_of[k] = slot
            dest_of[k, s_assign * SUPER_W + slot] = np.arange(rows_per_core)
    else:
        for k in range(ncores):
            sup_of[k] = np.arange(rows_per_core) // SUPER_W
            slot_of[k] = np.arange(rows_per_core) % SUPER_W
            dest_of[k, :rows_per_core] = np.arange(rows_per_core)

    sup = sup_of[core, r_local]
    slot = slot_of[core, r_local]
    gid = sup * N_CHUNKS + chunk
    n_groups = n_supers * N_CHUNKS

    counts = np.zeros((ncores, n_groups), np.int64)
    np.add.at(counts, (core, gid), 1)
    caps = counts.max(axis=0)
    caps = np.maximum(((caps + 127) // 128) * 128, 128).reshape(
        n_supers, N_CHUNKS)

    nbs = caps.sum(axis=1) // 128
    nbs_alloc = nbs + (nbs % 2)
    return (caps, nbs_alloc, counts, core, slot, sup, chunk, gid,
            n_supers, chunk_sz, dest_of)


def _pack_core(k, caps, nbs_alloc, counts, core, slot, sup, chunk, gid,
               edge_col, edge_val, chunk_sz):
    """Build per-core packed planes.

    Returns (idx_plane [128, IDX_COLS] int16, meta [128, META_COLS] fp16,
    cnts [1, n_groups] int32).
    idx: per (super, chunk) group, chunk-local cols wrapped in 16
    partitions, replicated 8x; pad = trailing -1 (ucode strips; the
    per-core true count goes in num_idxs_reg) or 0 when PAD_NEG1 is off.
    meta: per super, one-hot slot rows fp16 then vals fp16.
    """
    n_supers, n_chunks = caps.shape
    sel = np.flatnonzero(core == k)
    g = gid[sel]
    order = np.argsort(g, kind="stable")
    sel = sel[order]
    g = g[order]

    caps_flat = caps.reshape(-1)
    cap_off = np.zeros(n_supers * n_chunks + 1, np.int64)
    np.cumsum(caps_flat, out=cap_off[1:])
    grp_start = np.searchsorted(g, np.arange(n_supers * n_chunks))
    rank = np.arange(len(g)) - grp_start[g]
    pos = cap_off[g] + rank

    total = int(cap_off[-1])
    lc = np.full(total, -1 if PAD_NEG1 else 0, np.int16)
    rl = np.zeros(total, np.float16)
    vv = np.zeros(total, np.float16)
    lc[pos] = (edge_col[sel] - chunk[sel] * chunk_sz).astype(np.int16)
    rl[pos] = slot[sel].astype(np.float16)
    vv[pos] = edge_val[sel].astype(np.float16)

    idx_planes = []
    for gi in range(n_supers * n_chunks):
        a, b = int(cap_off[gi]), int(cap_off[gi + 1])
        cap = b - a
        w16 = lc[a:b].reshape(cap // 16, 16).T  # [16, cap/16]
        idx_planes.append(np.tile(w16, (8, 1)))  # [128, cap/16]
    idx_plane = np.ascontiguousarray(np.concatenate(idx_planes, axis=1))

    meta_planes = []
    for s in range(n_supers):
        a = int(cap_off[s * n_chunks])
        b = int(cap_off[(s + 1) * n_chunks])
        nb = (b - a) // 128
        nba = int(nbs_alloc[s])
        rows = np.zeros((128, nba), np.float16)
        vals = np.zeros((128, nba), np.float16)
        rows[:, :nb] = rl[a:b].reshape(nb, 128).T
        vals[:, :nb] = vv[a:b].reshape(nb, 128).T
        meta_planes.append(rows)
        meta_planes.append(vals)
    meta = np.ascontiguousarray(np.concatenate(meta_planes, axis=1))

    if PAD_NEG1:
        cnts = counts[k].astype(np.int32).reshape(1, -1)
    else:
        cnts = caps_flat.astype(np.int32).reshape(1, -1)
    return idx_plane, meta, np.ascontiguousarray(cnts)


def _build_program(caps, nbs_alloc, n_nodes, n_supers, chunk_sz,
                   mode="full"):
    import concourse.bacc as bacc
    import concourse.mybir as mybir
    import concourse.tile as tile

    _patch_tile_drain()

    fp16 = mybir.dt.float16
    fp32 = mybir.dt.float32
    int16 = mybir.dt.int16
    n_chunks = caps.shape[1]
    rows_pad = n_supers * SUPER_W
    nb_grp = caps // 128          # batches per (super, chunk)
    nba_max = int(nbs_alloc.max())
    msgs_nb_max = int(nb_grp.max())

    # idx column offsets per group; meta column offsets per super
    idx_off = np.zeros((n_supers, n_chunks), np.int64)
    o = 0
    for s in range(n_supers):
        for c in range(n_chunks):
            idx_off[s, c] = o
            o += int(caps[s, c]) // 16
    idx_cols = int(o)
    meta_off = np.zeros(n_supers, np.int64)
    o = 0
    for s in range(n_supers):
        meta_off[s] = o
        o += 2 * int(nbs_alloc[s])
    meta_cols = int(o)

    nc = bacc.Bacc("TRN2", target_bir_lowering=False,
                   num_swdge_queues=N_QUEUES)
    X16 = nc.dram_tensor("x16", [n_nodes, D], fp16, kind="ExternalInput")
    IDX = nc.dram_tensor("idx", [128, idx_cols], int16, kind="ExternalInput")
    META = nc.dram_tensor("meta", [128, meta_cols], fp16,
                          kind="ExternalInput")
    CNT = nc.dram_tensor("cnt", [1, n_supers * n_chunks], mybir.dt.int32,
                         kind="ExternalInput")
    IOTA = nc.dram_tensor("iota", [128, SUPER_W, nba_max], fp16,
                          kind="ExternalInput")
    IDENT = nc.dram_tensor("ident", [128, 128], fp16, kind="ExternalInput")
    WT = nc.dram_tensor("wt", [D, D], fp16, kind="ExternalInput")
    OUT = nc.dram_tensor("out", [rows_pad, D], fp32, kind="ExternalOutput")

    with tile.TileContext(nc) as tc, ExitStack() as ctx:
        const_pool = ctx.enter_context(tc.tile_pool(name="const", bufs=1))
        msgs_pool = ctx.enter_context(tc.tile_pool(name="msgs", bufs=14))
        oh_pool = ctx.enter_context(tc.tile_pool(name="oh", bufs=4))
        h_pool = ctx.enter_context(tc.tile_pool(name="h", bufs=2))
        ht_pool = ctx.enter_context(tc.tile_pool(name="ht", bufs=4))
        out_pool = ctx.enter_context(tc.tile_pool(name="outp", bufs=3))
        psum_pool = ctx.enter_context(
            tc.tile_pool(name="psum", bufs=3, space="PSUM"))
        psum_t_pool = ctx.enter_context(
            tc.tile_pool(name="psum_t", bufs=2, space="PSUM"))
        psum_o_pool = ctx.enter_context(
            tc.tile_pool(name="psum_o", bufs=2, space="PSUM"))

        idx_t = const_pool.tile([128, idx_cols], int16)
        nc.sync.dma_start(idx_t[:], IDX[:])
        meta_t = const_pool.tile([128, meta_cols], fp16)
        nc.sync.dma_start(meta_t[:], META[:])
        cnt_t = const_pool.tile([1, n_supers * n_chunks], mybir.dt.int32)
        nc.sync.dma_start(cnt_t[:], CNT[:])
        iota_t = const_pool.tile([128, SUPER_W, nba_max], fp16)
        nc.sync.dma_start(iota_t[:], IOTA[:])
        ident_t = const_pool.tile([128, 128], fp16)
        nc.sync.dma_start(ident_t[:], IDENT[:])
        wt_t = const_pool.tile([128, 2, D], fp16)
        nc.sync.dma_start(wt_t[:, 0, :], WT[0:128, :])
        nc.sync.dma_start(wt_t[:, 1, :], WT[128:256, :])

        for s in range(n_supers):
            nba = int(nbs_alloc[s])
            # --- batched one-hot for the whole super ---
            oh_t = oh_pool.tile([128, SUPER_W, nba_max], fp16, tag="oh")
            if mode == "noonehot":
                pass
            else:
                mo = int(meta_off[s])
                row_ap = meta_t[:, mo:mo + nba].unsqueeze(1).broadcast_to(
                    [128, SUPER_W, nba])
                val_ap = meta_t[:, mo + nba:mo + 2 * nba].unsqueeze(
                    1).broadcast_to([128, SUPER_W, nba])
                nc.vector.tensor_tensor(
                    oh_t[:, :, 0:nba], iota_t[:, :, 0:nba], row_ap,
                    mybir.AluOpType.is_equal)
                nc.vector.tensor_tensor(
                    oh_t[:, :, 0:nba], oh_t[:, :, 0:nba], val_ap,
                    mybir.AluOpType.mult)

            # --- gather + aggregate ---
            pT = psum_pool.tile([128, D], fp32, tag="ps")
            jj = 0
            first = True
            for c in range(n_chunks):
                cap = int(caps[s, c])
                nb = cap // 128
                mt = msgs_pool.tile([128, msgs_nb_max, D], fp16, tag="msgs")
                gi = s * n_chunks + c
                if gi < 14:
                    # first use of each msgs slot: clear the whole slot so
                    # pad rows (skipped by the short gather) always hold
                    # finite values for the val=0 one-hot columns
                    nc.vector.memset(mt[:], 0.0)
                if mode == "nogather":
                    pass
                else:
                    io = int(idx_off[s, c])

                    def _issue_gather(nreg, mt=mt, nb=nb, c=c, io=io,
                                      cap=cap):
                        nc.gpsimd.dma_gather(
                            mt[:, 0:nb, :],
                            X16[c * chunk_sz:(c + 1) * chunk_sz, :],
                            idx_t[:, io:io + cap // 16],
                            cap,
                            nreg,
                            D,
                            elem_step=D,
                            single_packet=(cap <= 1024),
                            queue_num=c % N_QUEUES,
                        )

                    if DYN_REG:
                        nreg = nc.gpsimd.value_load(cnt_t[0:1, gi:gi + 1])
                        _issue_gather(nreg)
                    else:
                        _issue_gather(cap)
                for j in range(nb):
                    last = (c == n_chunks - 1) and (j == nb - 1)
                    if mode == "nomm":
                        jj += 1
                        first = False
                        continue
                    nc.tensor.matmul(pT[:], oh_t[:, :, jj], mt[:, j, :],
                                     start=first, stop=last)
                    jj += 1
                    first = False
            if mode == "nomm":
                continue

            # --- linear: psum[d,f] -> hT -> @ W.T ---
            hs = h_pool.tile([128, D], fp16, tag="hs")
            nc.scalar.copy(hs[:], pT[:])
            po = psum_o_pool.tile([128, D], fp32, tag="po")
            if mode == "noW":
                ot = out_pool.tile([128, D], fp32, tag="ot")
                nc.scalar.copy(ot[:], hs[:])
            else:
                for hh in range(2):
                    ptr = psum_t_pool.tile([128, 128], fp16, tag="ptr")
                    nc.tensor.transpose(
                        ptr[:], hs[:, hh * 128:(hh + 1) * 128], ident_t[:])
                    ht = ht_pool.tile([128, 128], fp16, tag="ht")
                    nc.scalar.copy(ht[:], ptr[:])
                    nc.tensor.matmul(po[:], ht[:], wt_t[:, hh, :],
                                     start=(hh == 0), stop=(hh == 1))
                ot = out_pool.tile([128, D], fp32, tag="ot")
                nc.scalar.copy(ot[:], po[:])
            nc.sync.dma_start(
                OUT[s * SUPER_W:(s + 1) * SUPER_W, :], ot[:])
    nc.finalize()
    return nc


def _prepare(X, edge_row, edge_col, edge_val, W, mode="full"):
    X = np.asarray(X)
    edge_row = np.asarray(edge_row)
    edge_col = np.asarray(edge_col)
    edge_val = np.asarray(edge_val)
    W = np.asarray(W)

    (caps, nbs_alloc, counts, core, slot, sup, chunk, gid, n_supers,
     chunk_sz, dest_of) = _plan(edge_row, edge_col, N_NODES, NCORES)

    key = (mode, tuple(caps.reshape(-1).tolist()))
    if key not in _PROGRAM_CACHE:
        _PROGRAM_CACHE[key] = _build_program(
            caps, nbs_alloc, N_NODES, n_supers, chunk_sz, mode=mode)
    nc = _PROGRAM_CACHE[key]

    nba_max = int(nbs_alloc.max())
    X16 = np.ascontiguousarray(X.astype(np.float16))
    iota = np.ascontiguousarray(np.broadcast_to(
        np.arange(SUPER_W, dtype=np.float16)[None, :, None],
        (128, SUPER_W, nba_max)))
    ident = np.eye(128, dtype=np.float16)
    wt = np.ascontiguousarray(W.T.astype(np.float16))

    in_maps = []
    for k in range(NCORES):
        idx_plane, meta, cnts = _pack_core(
            k, caps, nbs_alloc, counts, core, slot, sup, chunk, gid,
            edge_col, edge_val, chunk_sz)
        in_maps.append({"x16": X16, "idx": idx_plane, "meta": meta,
                        "cnt": cnts, "iota": iota, "ident": ident,
                        "wt": wt})
    return nc, in_maps, dest_of


def _gather_out(res, b, dest_of):
    rows_per_core = N_NODES // NCORES
    out = np.empty((N_NODES, D), np.float32)
    for k in range(NCORES):
        o = res.results[k]["out"]  # [rows_pad, D], row sup*128+slot
        valid = dest_of[k] >= 0
        out[k * rows_per_core + dest_of[k, valid]] = o[valid]
    out += np.asarray(b).astype(np.float32)[None, :]
    return out


def kernel(X, edge_row, edge_col, edge_val, W, b):
    from concourse.bass_utils import run_bass_kernel_spmd

    nc, in_maps, dest_of = _prepare(X, edge_row, edge_col, edge_val, W)
    res = run_bass_kernel_spmd(nc, in_maps, core_ids=list(range(NCORES)))
    return _gather_out(res, b, dest_of)


def run_traced(X, edge_row, edge_col, edge_val, W, b, mode="full"):
    """Run with NTFF profiling; returns BassKernelResults."""
    from concourse.bass_utils import run_bass_kernel_spmd

    nc, in_maps, _ = _prepare(X, edge_row, edge_col, edge_val, W, mode=mode)
    return run_bass_kernel_spmd(nc, in_maps, core_ids=list(range(NCORES)),
                                trace=True)
